# revision 52
# baseline (speedup 1.0000x reference)
"""Sigmoid-attention block kernel for trn2 (one NeuronCore, SPMD over 8) — v15.

The host folds every H x H projection out of the device loop (float64):

  k'   = (key @ Wk.T + bk) @ Wq          -> shipped as k'T [H, N] bf16
  t    = key @ (Wk.T bq) + bq.bk         -> per-row logit offset [N] f32
  vP   = value @ Wv.T + bv               -> shipped as [N, H] bf16

so that logit_ij = k_j . q_i == k'_j . query_i + t_j with RAW query
(reassociation: k_j . (Wq query_i + bq) = (Wq.T k_j) . query_i + k_j . bq).
On-chip per core (q-slab of 1024 rows):

  attnT[j-block] = sigmoid(k'T-block.T @ queryT + t)   (bf16 MMs, fp32 PSUM)
  outT [H, SLAB] = sum_j vP[j].T-blocks @ attnT[j]     (bf16 MMs)

The PE stream is 512 back-to-back bf16 N=512 matmuls (~216 ns each at
2.4 GHz, the trn2 single-pump floor; ~90% of the kernel is this stream).
The t offset is applied half on the DVE (in-place PSUM add + plain
sigmoid) and half through the ACT bias port, keeping the ACT engine's
per-j work under the PE period; the out-accumulation matmuls trail the
logits by three j-periods so the sigmoid chain producing each at tile
has ~5 us of slack over its consumer. A memset-fed matmul warmup
bridges the framework preamble until the first k' chunk lands, so HAM
reaches 8/8 before real work. Output leaves as bf16 (error budget
~2e-3 << 2e-2) on both DMA rings.
"""
from contextlib import ExitStack

import concourse.bass as bass
import concourse.mybir as mybir
import concourse.tile as tile
from concourse import bacc

F32 = mybir.dt.float32
BF16 = mybir.dt.bfloat16
AF = mybir.ActivationFunctionType


def _build_attn_kernel(SLAB=1024, N=8192, H=256):
    assert H == 256
    NJ = N // 128            # 64 j-blocks (rows of attnT)
    ICW = 512                # i-chunk width
    IC = SLAB // ICW         # 2
    KCW = 512                # k'-chunk width = 4 j-blocks
    NKC = N // KCW           # 16
    JPC = KCW // 128         # 4
    HB = H // 128            # 2

    nc = bacc.Bacc()
    queryT = nc.dram_tensor("queryT", [H, SLAB], BF16, kind="ExternalInput")
    kPT = nc.dram_tensor("kPT", [H, N], BF16, kind="ExternalInput")
    valP = nc.dram_tensor("valP", [N, H], BF16, kind="ExternalInput")
    tT = nc.dram_tensor("tT", [128, NJ], F32, kind="ExternalInput")
    outd = nc.dram_tensor("outT", [H, SLAB], BF16, kind="ExternalOutput")

    with tile.TileContext(nc) as tc, ExitStack() as ctx:
        cpool = ctx.enter_context(tc.tile_pool(name="const", bufs=1))
        psW = ctx.enter_context(tc.tile_pool(name="psW", bufs=4, space="PSUM"))
        psO = ctx.enter_context(tc.tile_pool(name="psO", bufs=1, space="PSUM"))
        big = ctx.enter_context(tc.tile_pool(name="big", bufs=1))
        krot = ctx.enter_context(tc.tile_pool(name="krot", bufs=4))
        valp = ctx.enter_context(tc.tile_pool(name="valp", bufs=11))
        attnp = ctx.enter_context(tc.tile_pool(name="attnp", bufs=5))
        outp = ctx.enter_context(tc.tile_pool(name="outp", bufs=1))
        lgp = ctx.enter_context(tc.tile_pool(name="lgp", bufs=4))

        # ---- t / query loads on the scalar ring ----
        t_sb = cpool.tile([128, NJ], F32, tag="tT", name="tT")
        nc.scalar.dma_start(t_sb[:], tT[:, :])
        qu = []
        for hpb in range(HB):
            t = big.tile([128, SLAB], BF16, tag=f"qu{hpb}", name=f"qu{hpb}")
            nc.scalar.dma_start(t[:], queryT[hpb * 128:(hpb + 1) * 128, :])
            qu.append(t)

        # HAM warmup: a few bf16 matmuls on a memset tile keep the PE busy
        # while the head DMAs land, so the first real matmuls run at 2.4 GHz
        wu = cpool.tile([128, 512], BF16, tag="wu", name="wu")
        nc.gpsimd.memset(wu[:], 0.0)
        for _ in range(12):
            pw = psW.tile([128, 512], F32, tag="ps", name="pw")
            nc.tensor.matmul(pw[:], wu[:, :128], wu[:], start=True, stop=True)

        # ---- k' chunks + value rows stream in on the sync ring ----
        kch = {}

        def emit_kchunk_dma(c):
            tiles = []
            for hb in range(HB):
                t = krot.tile([128, KCW], BF16, tag=f"kch{hb}",
                              name=f"kch{hb}")
                nc.sync.dma_start(
                    t[:], kPT[hb * 128:(hb + 1) * 128, c * KCW:(c + 1) * KCW]
                )
                tiles.append(t)
            kch[c] = tiles

        val_t = [None] * NJ

        def emit_val_dma(j):
            t = valp.tile([128, H], BF16, tag="val")
            nc.sync.dma_start(t[:], valP[j * 128:(j + 1) * 128, :])
            val_t[j] = t

        emit_kchunk_dma(0)
        emit_kchunk_dma(1)
        emit_val_dma(0)
        emit_val_dma(1)
        emit_kchunk_dma(2)
        emit_val_dma(2)
        emit_kchunk_dma(3)
        for j in range(3, 6):
            emit_val_dma(j)

        at_tiles = [None] * NJ

        def emit_logits(j):
            kt = kch[j // JPC]
            jo = (j % JPC) * 128
            at = attnp.tile([128, SLAB], BF16, tag="at")
            for ic in range(IC):
                pl = psW.tile([128, 512], F32, tag="ps")
                for hb in range(HB):
                    nc.tensor.matmul(
                        pl[:],
                        kt[hb][:, jo:jo + 128],
                        qu[hb][:, ic * ICW:(ic + 1) * ICW],
                        start=(hb == 0),
                        stop=(hb == HB - 1),
                    )
                # DVE applies t while staging the logits out of PSUM into
                # an SBUF bf16 tile: the PSUM slot frees after the DVE read
                # (not after the sigmoid), killing the slot-reuse WAR that
                # stalled the logits matmuls; ACT then reads SBUF only
                lg = lgp.tile([128, 512], BF16, tag="lg")
                nc.vector.tensor_scalar_add(lg[:], pl[:], t_sb[:, j:j + 1])
                nc.scalar.activation(at[:, ic * ICW:(ic + 1) * ICW],
                                     lg[:], AF.Sigmoid)
            at_tiles[j] = at

        ps_o = [psO.tile([128, SLAB], F32, tag=f"po{hb}", name=f"po{hb}")
                for hb in range(HB)]

        def emit_oacc(j):
            at = at_tiles[j]
            # on the final accumulation, stop ps_o[1] first so its output
            # copies overlap ps_o[0]'s last matmuls
            hbs = [1, 0] if j == NJ - 1 else [0, 1]
            for hb in hbs:
                for ic in range(IC):
                    nc.tensor.matmul(
                        ps_o[hb][:, ic * ICW:(ic + 1) * ICW],
                        val_t[j][:, hb * 128:(hb + 1) * 128],
                        at[:, ic * ICW:(ic + 1) * ICW],
                        start=(j == 0),
                        stop=(j == NJ - 1),
                    )
            at_tiles[j] = None
            val_t[j] = None

        # ---- steady loop: oacc trails logits by TWO j-periods so the
        # sigmoid chain producing at(j) has ~3.5 us of slack over its
        # consumer (one period was ~140 ns short -> periodic PE stalls)
        j = 0
        for c in range(NKC):
            for _ in range(JPC):
                if j + 6 < NJ:
                    emit_val_dma(j + 6)
                emit_logits(j)
                if j >= 3:
                    emit_oacc(j - 3)
                j += 1
            kch.pop(c)
            if c + 4 < NKC:
                emit_kchunk_dma(c + 4)
        emit_oacc(NJ - 3)
        emit_oacc(NJ - 2)
        emit_oacc(NJ - 1)

        # ---- tail: ps_o[1] stops first; quarters alternate DVE/ACT copies
        # with a DMA ring per engine so the four chains run pairwise
        for hb, ic in ((1, 0), (1, 1), (0, 0), (0, 1)):
            o = outp.tile([128, ICW], BF16, tag=f"o{hb}{ic}",
                          name=f"o{hb}{ic}")
            src_ap = ps_o[hb][:, ic * ICW:(ic + 1) * ICW]
            if ic == 0:
                nc.vector.tensor_copy(o[:], src_ap)
                ring = nc.sync
            else:
                nc.scalar.activation(o[:], src_ap, AF.Copy)
                ring = nc.scalar
            ring.dma_start(
                outd[hb * 128:(hb + 1) * 128, ic * ICW:(ic + 1) * ICW],
                o[:],
            )

    nc.finalize()
    return nc


import numpy as np
import ml_dtypes
from concourse.bass_utils import run_bass_kernel_spmd

BF16_NP = ml_dtypes.bfloat16

N_CORES = 8
N_FULL = 8192
H_FULL = 256
SLAB_FULL = N_FULL // N_CORES

_NC = None


def _get_nc():
    global _NC
    if _NC is None:
        _NC = _build_attn_kernel(SLAB=SLAB_FULL, N=N_FULL, H=H_FULL)
    return _NC


def _in_maps(inputs):
    full = {k: np.asarray(v, dtype=np.float32) for k, v in inputs.items()}
    # Host folds (float64). With k = key@Wk.T + bk and q = query@Wq.T + bq:
    #   logit_ij = k_j . q_i = (k_j . Wq query_i) + (k_j . bq)
    #            = k'_j . query_i + t_j
    #   k' = k @ Wq,  t = k @ bq,  vP = value @ Wv.T + bv
    Wq64 = full["Wq"].astype(np.float64)
    k64 = (full["key"].astype(np.float64) @ full["Wk"].astype(np.float64).T
           + full["bk"].astype(np.float64))
    kP = k64 @ Wq64                                   # [N, H]
    t = (k64 @ full["bq"].astype(np.float64)).astype(np.float32)   # [N]
    vP = (full["value"].astype(np.float64) @ full["Wv"].astype(np.float64).T
          + full["bv"].astype(np.float64))
    queryT = np.ascontiguousarray(full["query"].T).astype(BF16_NP)  # [H, N]
    NJ = N_FULL // 128
    shared = {
        "kPT": np.ascontiguousarray(kP.T).astype(BF16_NP),
        "valP": np.ascontiguousarray(vP).astype(BF16_NP),
        "tT": np.ascontiguousarray(t.reshape(NJ, 128).T),
    }
    maps = []
    for c in range(N_CORES):
        m = dict(shared)
        m["queryT"] = np.ascontiguousarray(
            queryT[:, c * SLAB_FULL:(c + 1) * SLAB_FULL]
        )
        maps.append(m)
    return maps


def kernel(**inputs) -> np.ndarray:
    nc = _get_nc()
    res = run_bass_kernel_spmd(nc, _in_maps(inputs), list(range(N_CORES)))
    return np.ascontiguousarray(np.concatenate(
        [np.asarray(res.results[c]["outT"]).astype(np.float32).T
         for c in range(N_CORES)],
        axis=0,
    )).astype(np.float32)


# revision 54
# speedup vs baseline: 1.1995x; 1.1995x over previous
"""Sigmoid-attention block kernel for trn2 (one NeuronCore, SPMD over 8) — v15.

The host folds every H x H projection out of the device loop (float64):

  k'   = (key @ Wk.T + bk) @ Wq          -> shipped as k'T [H, N] bf16
  t    = key @ (Wk.T bq) + bq.bk         -> per-row logit offset [N] f32
  vP   = value @ Wv.T + bv               -> shipped as [N, H] bf16

so that logit_ij = k_j . q_i == k'_j . query_i + t_j with RAW query
(reassociation: k_j . (Wq query_i + bq) = (Wq.T k_j) . query_i + k_j . bq).
On-chip per core (q-slab of 1024 rows):

  attnT[j-block] = sigmoid(k'T-block.T @ queryT + t)   (bf16 MMs, fp32 PSUM)
  outT [H, SLAB] = sum_j vP[j].T-blocks @ attnT[j]     (bf16 MMs)

The PE stream is 512 back-to-back bf16 N=512 matmuls (~216 ns each at
2.4 GHz, the trn2 single-pump floor; ~90% of the kernel is this stream).
The t offset is applied half on the DVE (in-place PSUM add + plain
sigmoid) and half through the ACT bias port, keeping the ACT engine's
per-j work under the PE period so the 4-deep logits-PSUM rotation never
stalls the PE. A memset-fed matmul warmup bridges the framework preamble
until the first k' chunk lands, so HAM reaches 8/8 before real work.
Output leaves as bf16 (error budget ~2e-3 << 2e-2) on both DMA rings.
"""
from contextlib import ExitStack

import concourse.bass as bass
import concourse.mybir as mybir
import concourse.tile as tile
from concourse import bacc

F32 = mybir.dt.float32
BF16 = mybir.dt.bfloat16
AF = mybir.ActivationFunctionType


def _build_attn_kernel(SLAB=1024, N=8192, H=256):
    assert H == 256
    NJ = N // 128            # 64 j-blocks (rows of attnT)
    ICW = 512                # i-chunk width
    IC = SLAB // ICW         # 2
    KCW = 512                # k'-chunk width = 4 j-blocks
    NKC = N // KCW           # 16
    JPC = KCW // 128         # 4
    HB = H // 128            # 2

    nc = bacc.Bacc()
    queryT = nc.dram_tensor("queryT", [H, SLAB], BF16, kind="ExternalInput")
    kPT = nc.dram_tensor("kPT", [H, N], BF16, kind="ExternalInput")
    valP = nc.dram_tensor("valP", [N, H], BF16, kind="ExternalInput")
    tT = nc.dram_tensor("tT", [128, NJ], F32, kind="ExternalInput")
    outd = nc.dram_tensor("outT", [H, SLAB], BF16, kind="ExternalOutput")

    with tile.TileContext(nc) as tc, ExitStack() as ctx:
        cpool = ctx.enter_context(tc.tile_pool(name="const", bufs=1))
        psW = ctx.enter_context(tc.tile_pool(name="psW", bufs=4, space="PSUM"))
        psO = ctx.enter_context(tc.tile_pool(name="psO", bufs=1, space="PSUM"))
        big = ctx.enter_context(tc.tile_pool(name="big", bufs=1))
        krot = ctx.enter_context(tc.tile_pool(name="krot", bufs=4))
        valp = ctx.enter_context(tc.tile_pool(name="valp", bufs=11))
        attnp = ctx.enter_context(tc.tile_pool(name="attnp", bufs=5))
        outp = ctx.enter_context(tc.tile_pool(name="outp", bufs=1))
        lgp = ctx.enter_context(tc.tile_pool(name="lgp", bufs=4))

        # ---- t / query loads on the scalar ring ----
        t_sb = cpool.tile([128, NJ], F32, tag="tT", name="tT")
        nc.scalar.dma_start(t_sb[:], tT[:, :])
        qu = []
        for hpb in range(HB):
            t = big.tile([128, SLAB], BF16, tag=f"qu{hpb}", name=f"qu{hpb}")
            nc.scalar.dma_start(t[:], queryT[hpb * 128:(hpb + 1) * 128, :])
            qu.append(t)

        # HAM warmup: a few bf16 matmuls on a memset tile keep the PE busy
        # while the head DMAs land, so the first real matmuls run at 2.4 GHz
        wu = cpool.tile([128, 512], BF16, tag="wu", name="wu")
        nc.gpsimd.memset(wu[:], 0.0)
        for _ in range(12):
            pw = psW.tile([128, 512], F32, tag="ps", name="pw")
            nc.tensor.matmul(pw[:], wu[:, :128], wu[:], start=True, stop=True)

        # ---- k' chunks + value rows stream in on the sync ring ----
        kch = {}

        def emit_kchunk_dma(c):
            tiles = []
            for hb in range(HB):
                t = krot.tile([128, KCW], BF16, tag=f"kch{hb}",
                              name=f"kch{hb}")
                nc.sync.dma_start(
                    t[:], kPT[hb * 128:(hb + 1) * 128, c * KCW:(c + 1) * KCW]
                )
                tiles.append(t)
            kch[c] = tiles

        val_t = [None] * NJ

        def emit_val_dma(j):
            t = valp.tile([128, H], BF16, tag="val")
            nc.sync.dma_start(t[:], valP[j * 128:(j + 1) * 128, :])
            val_t[j] = t

        emit_kchunk_dma(0)
        emit_kchunk_dma(1)
        emit_val_dma(0)
        emit_val_dma(1)
        emit_kchunk_dma(2)
        emit_val_dma(2)
        emit_kchunk_dma(3)
        for j in range(3, 6):
            emit_val_dma(j)

        at_tiles = [None] * NJ

        def emit_logits(j):
            kt = kch[j // JPC]
            jo = (j % JPC) * 128
            at = attnp.tile([128, SLAB], BF16, tag="at")
            for ic in range(IC):
                pl = psW.tile([128, 512], F32, tag="ps")
                for hb in range(HB):
                    nc.tensor.matmul(
                        pl[:],
                        kt[hb][:, jo:jo + 128],
                        qu[hb][:, ic * ICW:(ic + 1) * ICW],
                        start=(hb == 0),
                        stop=(hb == HB - 1),
                    )
                # DVE applies t while staging the logits out of PSUM into
                # an SBUF bf16 tile: the PSUM slot frees after the DVE read
                # and the sigmoid reads SBUF only
                lg = lgp.tile([128, 512], BF16, tag="lg")
                nc.vector.tensor_scalar_add(lg[:], pl[:], t_sb[:, j:j + 1])
                nc.scalar.activation(at[:, ic * ICW:(ic + 1) * ICW],
                                     lg[:], AF.Sigmoid)
            at_tiles[j] = at

        ps_o = [psO.tile([128, SLAB], F32, tag=f"po{hb}", name=f"po{hb}")
                for hb in range(HB)]

        def emit_oacc(j):
            at = at_tiles[j]
            # on the final accumulation, stop ps_o[1] first so its output
            # copies overlap ps_o[0]'s last matmuls
            hbs = [1, 0] if j == NJ - 1 else [0, 1]
            for hb in hbs:
                for ic in range(IC):
                    nc.tensor.matmul(
                        ps_o[hb][:, ic * ICW:(ic + 1) * ICW],
                        val_t[j][:, hb * 128:(hb + 1) * 128],
                        at[:, ic * ICW:(ic + 1) * ICW],
                        start=(j == 0),
                        stop=(j == NJ - 1),
                    )
            at_tiles[j] = None
            val_t[j] = None

        # ---- steady loop: oacc trails logits by TWO j-periods so the
        # sigmoid chain producing at(j) has ~3.5 us of slack over its
        # consumer (one period was ~140 ns short -> periodic PE stalls)
        j = 0
        for c in range(NKC):
            for _ in range(JPC):
                if j + 6 < NJ:
                    emit_val_dma(j + 6)
                emit_logits(j)
                if j >= 3:
                    emit_oacc(j - 3)
                j += 1
            kch.pop(c)
            if c + 4 < NKC:
                emit_kchunk_dma(c + 4)
        emit_oacc(NJ - 3)
        emit_oacc(NJ - 2)
        emit_oacc(NJ - 1)

        # ---- tail: ps_o[1] stops first; quarters alternate DVE/ACT copies
        # with a DMA ring per engine so the four chains run pairwise
        for hb, ic in ((1, 0), (1, 1), (0, 0), (0, 1)):
            o = outp.tile([128, ICW], BF16, tag=f"o{hb}{ic}",
                          name=f"o{hb}{ic}")
            src_ap = ps_o[hb][:, ic * ICW:(ic + 1) * ICW]
            if ic == 0:
                nc.vector.tensor_copy(o[:], src_ap)
                ring = nc.sync
            else:
                nc.scalar.activation(o[:], src_ap, AF.Copy)
                ring = nc.scalar
            ring.dma_start(
                outd[hb * 128:(hb + 1) * 128, ic * ICW:(ic + 1) * ICW],
                o[:],
            )

    nc.finalize()
    return nc


import numpy as np
import ml_dtypes
from concourse.bass_utils import run_bass_kernel_spmd

BF16_NP = ml_dtypes.bfloat16

N_CORES = 8
N_FULL = 8192
H_FULL = 256
SLAB_FULL = N_FULL // N_CORES

_NC = None


def _get_nc():
    global _NC
    if _NC is None:
        _NC = _build_attn_kernel(SLAB=SLAB_FULL, N=N_FULL, H=H_FULL)
    return _NC


def _in_maps(inputs):
    full = {k: np.asarray(v, dtype=np.float32) for k, v in inputs.items()}
    # Host folds (float64). With k = key@Wk.T + bk and q = query@Wq.T + bq:
    #   logit_ij = k_j . q_i = (k_j . Wq query_i) + (k_j . bq)
    #            = k'_j . query_i + t_j
    #   k' = k @ Wq,  t = k @ bq,  vP = value @ Wv.T + bv
    Wq64 = full["Wq"].astype(np.float64)
    k64 = (full["key"].astype(np.float64) @ full["Wk"].astype(np.float64).T
           + full["bk"].astype(np.float64))
    kP = k64 @ Wq64                                   # [N, H]
    t = (k64 @ full["bq"].astype(np.float64)).astype(np.float32)   # [N]
    vP = (full["value"].astype(np.float64) @ full["Wv"].astype(np.float64).T
          + full["bv"].astype(np.float64))
    queryT = np.ascontiguousarray(full["query"].T).astype(BF16_NP)  # [H, N]
    NJ = N_FULL // 128
    shared = {
        "kPT": np.ascontiguousarray(kP.T).astype(BF16_NP),
        "valP": np.ascontiguousarray(vP).astype(BF16_NP),
        "tT": np.ascontiguousarray(t.reshape(NJ, 128).T),
    }
    maps = []
    for c in range(N_CORES):
        m = dict(shared)
        m["queryT"] = np.ascontiguousarray(
            queryT[:, c * SLAB_FULL:(c + 1) * SLAB_FULL]
        )
        maps.append(m)
    return maps


def kernel(**inputs) -> np.ndarray:
    nc = _get_nc()
    res = run_bass_kernel_spmd(nc, _in_maps(inputs), list(range(N_CORES)))
    return np.ascontiguousarray(np.concatenate(
        [np.asarray(res.results[c]["outT"]).astype(np.float32).T
         for c in range(N_CORES)],
        axis=0,
    )).astype(np.float32)


# revision 56
# speedup vs baseline: 1.2055x; 1.0050x over previous
"""Sigmoid-attention block kernel for trn2 (one NeuronCore, SPMD over 8) — v15.

The host folds every H x H projection out of the device loop (float64):

  k'   = (key @ Wk.T + bk) @ Wq          -> shipped as k'T [H, N] bf16
  t    = key @ (Wk.T bq) + bq.bk         -> per-row logit offset [N] f32
  vP   = value @ Wv.T + bv               -> shipped as [N, H] bf16

so that logit_ij = k_j . q_i == k'_j . query_i + t_j with RAW query
(reassociation: k_j . (Wq query_i + bq) = (Wq.T k_j) . query_i + k_j . bq).
On-chip per core (q-slab of 1024 rows):

  attnT[j-block] = sigmoid(k'T-block.T @ queryT + t)   (bf16 MMs, fp32 PSUM)
  outT [H, SLAB] = sum_j vP[j].T-blocks @ attnT[j]     (bf16 MMs)

The PE stream is 512 back-to-back bf16 N=512 matmuls (~216 ns each at
2.4 GHz, the trn2 single-pump floor; ~90% of the kernel is this stream).
The t offset is applied half on the DVE (in-place PSUM add + plain
sigmoid) and half through the ACT bias port, keeping the ACT engine's
per-j work under the PE period so the 4-deep logits-PSUM rotation never
stalls the PE. A memset-fed matmul warmup bridges the framework preamble
until the first k' chunk lands, so HAM reaches 8/8 before real work.
Output leaves as bf16 (error budget ~2e-3 << 2e-2) on both DMA rings.
"""
from contextlib import ExitStack

import concourse.bass as bass
import concourse.mybir as mybir
import concourse.tile as tile
from concourse import bacc

F32 = mybir.dt.float32
BF16 = mybir.dt.bfloat16
AF = mybir.ActivationFunctionType


def _build_attn_kernel(SLAB=1024, N=8192, H=256):
    assert H == 256
    NJ = N // 128            # 64 j-blocks (rows of attnT)
    ICW = 512                # i-chunk width
    IC = SLAB // ICW         # 2
    KCW = 512                # k'-chunk width = 4 j-blocks
    NKC = N // KCW           # 16
    JPC = KCW // 128         # 4
    HB = H // 128            # 2

    nc = bacc.Bacc()
    queryT = nc.dram_tensor("queryT", [H, SLAB], BF16, kind="ExternalInput")
    kPT = nc.dram_tensor("kPT", [H, N], BF16, kind="ExternalInput")
    valP = nc.dram_tensor("valP", [N, H], BF16, kind="ExternalInput")
    tT = nc.dram_tensor("tT", [128, NJ], F32, kind="ExternalInput")
    outd = nc.dram_tensor("outT", [H, SLAB], BF16, kind="ExternalOutput")

    with tile.TileContext(nc) as tc, ExitStack() as ctx:
        cpool = ctx.enter_context(tc.tile_pool(name="const", bufs=1))
        psW = ctx.enter_context(tc.tile_pool(name="psW", bufs=4, space="PSUM"))
        psO = ctx.enter_context(tc.tile_pool(name="psO", bufs=1, space="PSUM"))
        big = ctx.enter_context(tc.tile_pool(name="big", bufs=1))
        krot = ctx.enter_context(tc.tile_pool(name="krot", bufs=4))
        valp = ctx.enter_context(tc.tile_pool(name="valp", bufs=11))
        attnp = ctx.enter_context(tc.tile_pool(name="attnp", bufs=5))
        outp = ctx.enter_context(tc.tile_pool(name="outp", bufs=1))

        # ---- t / query loads on the scalar ring ----
        t_sb = cpool.tile([128, NJ], F32, tag="tT", name="tT")
        nc.scalar.dma_start(t_sb[:], tT[:, :])
        qu = []
        for hpb in range(HB):
            t = big.tile([128, SLAB], BF16, tag=f"qu{hpb}", name=f"qu{hpb}")
            nc.scalar.dma_start(t[:], queryT[hpb * 128:(hpb + 1) * 128, :])
            qu.append(t)

        # HAM warmup: a few bf16 matmuls on a memset tile keep the PE busy
        # while the head DMAs land, so the first real matmuls run at 2.4 GHz
        wu = cpool.tile([128, 512], BF16, tag="wu", name="wu")
        nc.gpsimd.memset(wu[:], 0.0)
        for _ in range(12):
            pw = psW.tile([128, 512], F32, tag="ps", name="pw")
            nc.tensor.matmul(pw[:], wu[:, :128], wu[:], start=True, stop=True)

        # ---- k' chunks + value rows stream in on the sync ring ----
        kch = {}

        def emit_kchunk_dma(c):
            tiles = []
            for hb in range(HB):
                t = krot.tile([128, KCW], BF16, tag=f"kch{hb}",
                              name=f"kch{hb}")
                nc.sync.dma_start(
                    t[:], kPT[hb * 128:(hb + 1) * 128, c * KCW:(c + 1) * KCW]
                )
                tiles.append(t)
            kch[c] = tiles

        val_t = [None] * NJ

        def emit_val_dma(j):
            t = valp.tile([128, H], BF16, tag="val")
            nc.sync.dma_start(t[:], valP[j * 128:(j + 1) * 128, :])
            val_t[j] = t

        emit_kchunk_dma(0)
        emit_kchunk_dma(1)
        emit_val_dma(0)
        emit_val_dma(1)
        emit_kchunk_dma(2)
        emit_val_dma(2)
        emit_kchunk_dma(3)
        for j in range(3, 6):
            emit_val_dma(j)

        at_tiles = [None] * NJ

        def emit_logits(j):
            kt = kch[j // JPC]
            jo = (j % JPC) * 128
            at = attnp.tile([128, SLAB], BF16, tag="at")
            for ic in range(IC):
                pl = psW.tile([128, 512], F32, tag="ps")
                for hb in range(HB):
                    nc.tensor.matmul(
                        pl[:],
                        kt[hb][:, jo:jo + 128],
                        qu[hb][:, ic * ICW:(ic + 1) * ICW],
                        start=(hb == 0),
                        stop=(hb == HB - 1),
                    )
                # ic0: t via the ACT bias port (at-half ready ~750 ns
                # earlier); ic1: t-add on the idle DVE + plain sigmoid.
                # Splitting keeps both engines under the PE pace.
                if ic == 0:
                    nc.scalar.activation(at[:, ic * ICW:(ic + 1) * ICW],
                                         pl[:], AF.Sigmoid,
                                         bias=t_sb[:, j:j + 1])
                else:
                    nc.vector.tensor_scalar_add(pl[:], pl[:], t_sb[:, j:j + 1])
                    nc.scalar.activation(at[:, ic * ICW:(ic + 1) * ICW],
                                         pl[:], AF.Sigmoid)
            at_tiles[j] = at

        ps_o = [psO.tile([128, SLAB], F32, tag=f"po{hb}", name=f"po{hb}")
                for hb in range(HB)]

        def emit_oacc(j):
            at = at_tiles[j]
            # on the final accumulation, stop ps_o[1] first so its output
            # copies overlap ps_o[0]'s last matmuls
            hbs = [1, 0] if j == NJ - 1 else [0, 1]
            for hb in hbs:
                for ic in range(IC):
                    nc.tensor.matmul(
                        ps_o[hb][:, ic * ICW:(ic + 1) * ICW],
                        val_t[j][:, hb * 128:(hb + 1) * 128],
                        at[:, ic * ICW:(ic + 1) * ICW],
                        start=(j == 0),
                        stop=(j == NJ - 1),
                    )
            at_tiles[j] = None
            val_t[j] = None

        # ---- steady loop: oacc trails logits by TWO j-periods so the
        # sigmoid chain producing at(j) has ~3.5 us of slack over its
        # consumer (one period was ~140 ns short -> periodic PE stalls)
        j = 0
        for c in range(NKC):
            for _ in range(JPC):
                if j + 6 < NJ:
                    emit_val_dma(j + 6)
                emit_logits(j)
                if j >= 3:
                    emit_oacc(j - 3)
                j += 1
            kch.pop(c)
            if c + 4 < NKC:
                emit_kchunk_dma(c + 4)
        emit_oacc(NJ - 3)
        emit_oacc(NJ - 2)
        emit_oacc(NJ - 1)

        # ---- tail: ps_o[1] stops first; quarters alternate DVE/ACT copies
        # with a DMA ring per engine so the four chains run pairwise
        for hb, ic in ((1, 0), (1, 1), (0, 0), (0, 1)):
            o = outp.tile([128, ICW], BF16, tag=f"o{hb}{ic}",
                          name=f"o{hb}{ic}")
            src_ap = ps_o[hb][:, ic * ICW:(ic + 1) * ICW]
            if ic == 0:
                nc.vector.tensor_copy(o[:], src_ap)
                ring = nc.sync
            else:
                nc.scalar.activation(o[:], src_ap, AF.Copy)
                ring = nc.scalar
            ring.dma_start(
                outd[hb * 128:(hb + 1) * 128, ic * ICW:(ic + 1) * ICW],
                o[:],
            )

    nc.finalize()
    return nc


import numpy as np
import ml_dtypes
from concourse.bass_utils import run_bass_kernel_spmd

BF16_NP = ml_dtypes.bfloat16

N_CORES = 8
N_FULL = 8192
H_FULL = 256
SLAB_FULL = N_FULL // N_CORES

_NC = None


def _get_nc():
    global _NC
    if _NC is None:
        _NC = _build_attn_kernel(SLAB=SLAB_FULL, N=N_FULL, H=H_FULL)
    return _NC


def _in_maps(inputs):
    full = {k: np.asarray(v, dtype=np.float32) for k, v in inputs.items()}
    # Host folds (float64). With k = key@Wk.T + bk and q = query@Wq.T + bq:
    #   logit_ij = k_j . q_i = (k_j . Wq query_i) + (k_j . bq)
    #            = k'_j . query_i + t_j
    #   k' = k @ Wq,  t = k @ bq,  vP = value @ Wv.T + bv
    Wq64 = full["Wq"].astype(np.float64)
    k64 = (full["key"].astype(np.float64) @ full["Wk"].astype(np.float64).T
           + full["bk"].astype(np.float64))
    kP = k64 @ Wq64                                   # [N, H]
    t = (k64 @ full["bq"].astype(np.float64)).astype(np.float32)   # [N]
    vP = (full["value"].astype(np.float64) @ full["Wv"].astype(np.float64).T
          + full["bv"].astype(np.float64))
    queryT = np.ascontiguousarray(full["query"].T).astype(BF16_NP)  # [H, N]
    NJ = N_FULL // 128
    shared = {
        "kPT": np.ascontiguousarray(kP.T).astype(BF16_NP),
        "valP": np.ascontiguousarray(vP).astype(BF16_NP),
        "tT": np.ascontiguousarray(t.reshape(NJ, 128).T),
    }
    maps = []
    for c in range(N_CORES):
        m = dict(shared)
        m["queryT"] = np.ascontiguousarray(
            queryT[:, c * SLAB_FULL:(c + 1) * SLAB_FULL]
        )
        maps.append(m)
    return maps


def kernel(**inputs) -> np.ndarray:
    nc = _get_nc()
    res = run_bass_kernel_spmd(nc, _in_maps(inputs), list(range(N_CORES)))
    return np.ascontiguousarray(np.concatenate(
        [np.asarray(res.results[c]["outT"]).astype(np.float32).T
         for c in range(N_CORES)],
        axis=0,
    )).astype(np.float32)
